# revision 10
# baseline (speedup 1.0000x reference)
"""Grouped SwiGLU expert MLP (MoE) on 8 Trainium2 NeuronCores.

Problem: sorted_x [32768, 512] f32, tokens pre-sorted by expert into 8 equal
contiguous segments of 4096 tokens; per-expert SwiGLU MLP
    h12 = x_e @ w12[e].T          (4096, 2816)
    h   = silu(h12[:, :1408]) * h12[:, 1408:]
    out = h @ w3[e].T             (4096, 512)

Sharding: pure expert parallelism — core e owns expert e's weights and its
4096-token segment (sliced host-side from expert_starts), so no device-side
collectives are needed; the host concatenates the per-core outputs.

Device layout is feature-major throughout ("contraction dim on partitions"),
which makes both GEMMs transpose-free on chip:
    xt   = x_e.T   [512, 4096]  fp16
    w12t = w12.T   [512, 2816]  fp16
    w3t  = w3.T    [1408, 512]  fp16
    outT = out.T   [512, 4096]  f32   (host transposes back)
GEMM1 produces H12^T tiles [128h, Nt] (PSUM), SwiGLU runs on ACT+DVE into
fp16 H^T tiles, GEMM2 consumes them directly. fp16 operands run the PE at
1 cycle/row (vs 4 for f32); accumulation is always f32 in PSUM.

The kernel is PE-bound: 540672 matmul rows = 225us at 2.4GHz, so the
schedule's whole job is keeping the PE gap-free from first matmul to the
end. Measured fixed costs that shape the schedule: ~6us engine preamble,
~2.6us DMA ring-fetch latency from issue to data, ~0.7us semaphore lag,
~8.7us end-of-context closure.
  - Blocks: six full 512-token blocks then two 256-token blocks, so the
    final flush/copy/store drain is short. Block 0's hh=0 GEMM1 is split
    into two 256-token halves so the PE starts as soon as the first 256
    tokens (x0a) land instead of waiting for all 512.
  - DMA issue order == compute-need order. Everything the PE needs in the
    first ~20us rides the two HWDGE rings, whose completion semaphores
    post promptly: sync carries the w12 gate/up chunks in ascending-hh
    order plus the in-loop stores, scalar carries x0a/x0b and the early
    w3 chunks. The SWDGE (gpsimd) Q7 core starves its semaphore posting
    while it streams descriptor generations, so it gets only bulk loads
    that are needed late (x blocks 1-7, w3 kh>=6).
  - The last block's stores go as two do-pair DMAs on sync + scalar (NOT
    gpsimd: the SWDGE ring also takes a ~4us software drain after its
    last DMA, which would land on the closure path).
"""

import os

import numpy as np
import ml_dtypes

import concourse.bass as bass
import concourse.mybir as mybir
import concourse.tile as tile
from concourse import bacc
from concourse.bass_utils import run_bass_kernel_spmd

N_CORES = 8
D = 512  # d_model
H = 1408  # hidden
TWOH = 2 * H
TPE = 4096  # tokens per expert
NT = 512  # full token block (one PSUM bank in f32)
KD = D // 128  # 4 contraction tiles over d
KH = H // 128  # 11 contraction tiles over h

# (start, size) token blocks: full middles, small tail (see docstring)
BLOCKS = [(512 * b, 512) for b in range(7)] + [(3584, 256), (3840, 256)]

F16 = mybir.dt.float16
F32 = mybir.dt.float32
NP_F16 = np.dtype(np.float16)

# Results of a traced run (test harness reads these).
last_exec_time_ns = None
last_trace_path = None


def _build():
    # Bacc (not plain Bass): its compile() pass pipeline legalizes sync
    # waits (>=2 waits per instruction are split into event-sem chains),
    # which this image's walrus requires.
    nc = bacc.Bacc("TRN2", target_bir_lowering=False, debug=False, num_devices=N_CORES)
    xt = nc.dram_tensor("xt", [D, TPE], F16, kind="ExternalInput")
    w12t = nc.dram_tensor("w12t", [D, TWOH], F16, kind="ExternalInput")
    w3t = nc.dram_tensor("w3t", [H, D], F16, kind="ExternalInput")
    outT = nc.dram_tensor("outT", [D, TPE], F32, kind="ExternalOutput")

    # GEMM2 is software-pipelined into the GEMM1/SwiGLU loop with this lag:
    # in iteration hh we issue the GEMM2 matmuls consuming ht[hh - LAG], so
    # the PE never waits on the ACT+DVE SwiGLU chain.
    LAG = 2

    with tile.TileContext(nc) as tc:
        with (
            tc.tile_pool(name="weights", bufs=1) as wpool,
            tc.tile_pool(name="xin", bufs=1) as xpool,
            tc.tile_pool(name="ht", bufs=2) as hpool,
            tc.tile_pool(name="swi", bufs=4) as spool,
            tc.tile_pool(name="ot", bufs=4) as opool,
            tc.tile_pool(name="pg", bufs=2, space=bass.MemorySpace.PSUM) as pgate,
            tc.tile_pool(name="pu", bufs=2, space=bass.MemorySpace.PSUM) as pup,
            tc.tile_pool(name="po", bufs=1, space=bass.MemorySpace.PSUM) as pacc,
        ):
            w12s = wpool.tile([128, KD, TWOH], F16)
            w3s = wpool.tile([128, KH, D], F16)
            xs = xpool.tile([128, KD, TPE], F16)

            xt_r = xt[:, :].rearrange("(kd p) t -> p kd t", p=128)
            w12_r = w12t[:, :].rearrange("(kd p) h -> p kd h", p=128)
            w3_r = w3t[:, :].rearrange("(kh p) d -> p kh d", p=128)

            def dma_w12(c0, c1, eng=None):
                (eng or nc.gpsimd).dma_start(
                    out=w12s[:, :, c0:c1], in_=w12_r[:, :, c0:c1]
                )

            def dma_x(t0, t1, eng):
                eng.dma_start(out=xs[:, :, t0:t1], in_=xt_r[:, :, t0:t1])

            # DMA issue order == compute-need order (see module docstring).
            # Ring constraints (measured): HWDGE rings (sync/scalar) post
            # completion sems promptly but hold only ~2-3 DMAs of
            # descriptors before the issuing sequencer backpressures, so
            # each gets a short early burst. The SWDGE (gpsimd) Q7 posts
            # sems only once its generation queue drains (~0.9us/DMA), so
            # it carries only chunks needed after ~+15us, with the bulk
            # token loads last so they trail the weights within the ring.
            dma_w12(0, 128, nc.scalar)  # gate hh=0: first-matmul critical
            dma_x(0, 256, nc.sync)  # x0a: with it the split hh=0 can start
            dma_w12(H, H + 128, nc.scalar)  # up hh=0
            dma_x(256, 512, nc.sync)
            dma_w12(H + 128, H + 256, nc.scalar)  # up hh=1
            dma_w12(128, 256, nc.sync)  # gate hh=1
            dma_w12(256, 512, nc.sync)  # gate hh=2-3
            nc.sync.dma_start(out=w3s[:, 0:1, :], in_=w3_r[:, 0:1, :])
            dma_w12(H + 256, H + 512)  # up hh=2-3 heads the gpsimd ring
            dma_w12(512, 896)
            dma_w12(H + 512, H + 896)
            dma_w12(896, 1408)
            dma_w12(H + 896, H + 1408)
            nc.sync.dma_start(out=w3s[:, 1:6, :], in_=w3_r[:, 1:6, :])
            nc.sync.dma_start(out=w3s[:, 6:KH, :], in_=w3_r[:, 6:KH, :])
            dma_x(512, 1024, nc.sync)
            # Bulk token blocks trail the weight stream on the gpsimd ring.
            dma_x(1024, 2560, nc.gpsimd)
            dma_x(2560, 4096, nc.gpsimd)

            outT_r = outT[:, :].rearrange("(do p) t -> p do t", p=128)

            for tb, (ts0, bs) in enumerate(BLOCKS):
                tsl = slice(ts0, ts0 + bs)
                ht = hpool.tile([128, KH, NT], F16)
                acc = [
                    pacc.tile([128, NT], F32, name=f"acc{do}", tag=f"acc{do}")
                    for do in range(KD)
                ]

                def gemm2_step(kh):
                    for do in range(KD):
                        nc.tensor.matmul(
                            acc[do][:, :bs],
                            w3s[:, kh, do * 128 : (do + 1) * 128],
                            ht[:, kh, :bs],
                            start=(kh == 0),
                            stop=(kh == KH - 1),
                        )

                for hh in range(KH):
                    ps_g = pgate.tile([128, NT], F32)
                    ps_u = pup.tile([128, NT], F32)
                    # Block 0, hh=0: run GEMM1 in two 256-token halves so
                    # the first matmuls only need x0a (not all of x block 0)
                    if tb == 0 and hh == 0:
                        tranges = [(0, 256), (256, 512)]
                    else:
                        tranges = [(0, bs)]
                    for ps, coff in ((ps_g, 0), (ps_u, H)):
                        for ta, tz in tranges:
                            for kd in range(KD):
                                nc.tensor.matmul(
                                    ps[:, ta:tz],
                                    w12s[:, kd, coff + hh * 128 : coff + (hh + 1) * 128],
                                    xs[:, kd, ts0 + ta : ts0 + tz],
                                    start=(kd == 0),
                                    stop=(kd == KD - 1),
                                )
                    sil = spool.tile([128, NT], F32)
                    nc.scalar.activation(
                        sil[:, :bs], ps_g[:, :bs], mybir.ActivationFunctionType.Silu
                    )
                    nc.vector.tensor_mul(ht[:, hh, :bs], sil[:, :bs], ps_u[:, :bs])
                    if hh >= LAG:
                        gemm2_step(hh - LAG)
                for kh in range(KH - LAG, KH):
                    gemm2_step(kh)

                # PSUM->SBUF copies split across ACT and DVE; one coalesced
                # output DMA per block, except the last block where two
                # do-pair stores issue on sync + scalar in parallel.
                ot = opool.tile([128, KD, NT], F32)
                for do in range(KD):
                    if do % 2 == 0:
                        nc.scalar.copy(ot[:, do, :bs], acc[do][:, :bs])
                    else:
                        nc.vector.tensor_copy(ot[:, do, :bs], acc[do][:, :bs])
                if tb < len(BLOCKS) - 1:
                    nc.sync.dma_start(out=outT_r[:, :, tsl], in_=ot[:, :, :bs])
                else:
                    nc.sync.dma_start(out=outT_r[:, 0:2, tsl], in_=ot[:, 0:2, :bs])
                    nc.scalar.dma_start(
                        out=outT_r[:, 2:4, tsl], in_=ot[:, 2:4, :bs]
                    )
    nc.compile()
    return nc


_nc_cache = None


def _get_nc():
    global _nc_cache
    if _nc_cache is None:
        _nc_cache = _build()
    return _nc_cache


def kernel(sorted_x, w12, w3, expert_starts, expert_ends):
    global last_exec_time_ns, last_trace_path
    sorted_x = np.asarray(sorted_x)
    w12 = np.asarray(w12)
    w3 = np.asarray(w3)
    starts = np.asarray(expert_starts).astype(np.int64)
    T = sorted_x.shape[0]

    in_maps = []
    for e in range(N_CORES):
        # jax.lax.dynamic_slice clamps the start index the same way
        s = int(min(max(starts[e], 0), T - TPE))
        xe = sorted_x[s : s + TPE]  # (TPE, D) f32
        in_maps.append(
            {
                "xt": np.ascontiguousarray(xe.T).astype(NP_F16),
                "w12t": np.ascontiguousarray(w12[e].T).astype(NP_F16),
                "w3t": np.ascontiguousarray(w3[e].T).astype(NP_F16),
            }
        )

    trace = bool(os.environ.get("BASS_MOE_TRACE"))
    res = run_bass_kernel_spmd(
        _get_nc(), in_maps, core_ids=list(range(N_CORES)), trace=trace
    )
    if trace:
        last_exec_time_ns = res.exec_time_ns
        iat = res.instructions_and_trace
        last_trace_path = iat[1] if iat else None

    out = np.empty((N_CORES * TPE, D), dtype=np.float32)
    for e in range(N_CORES):
        out[e * TPE : (e + 1) * TPE] = res.results[e]["outT"].T
    return out


# revision 13
# speedup vs baseline: 1.2383x; 1.2383x over previous
"""Grouped SwiGLU expert MLP (MoE) on 8 Trainium2 NeuronCores.

Problem: sorted_x [32768, 512] f32, tokens pre-sorted by expert into 8 equal
contiguous segments of 4096 tokens; per-expert SwiGLU MLP
    h12 = x_e @ w12[e].T          (4096, 2816)
    h   = silu(h12[:, :1408]) * h12[:, 1408:]
    out = h @ w3[e].T             (4096, 512)

Sharding: pure expert parallelism — core e owns expert e's weights and its
4096-token segment (sliced host-side from expert_starts), so no device-side
collectives are needed; the host concatenates the per-core outputs.

Device layout is feature-major throughout ("contraction dim on partitions"),
which makes both GEMMs transpose-free on chip:
    xt   = x_e.T   [512, 4096]  fp16
    w12t = w12.T   [512, 2816]  fp16
    w3t  = w3.T    [1408, 512]  fp16
    outT = out.T   [512, 4096]  f32   (host transposes back)
GEMM1 produces H12^T tiles [128h, Nt] (PSUM), SwiGLU runs on ACT+DVE into
fp16 H^T tiles, GEMM2 consumes them directly. fp16 operands run the PE at
1 cycle/row (vs 4 for f32); accumulation is always f32 in PSUM.

The kernel is PE-bound: 540672 matmul rows = 225us at 2.4GHz, so the
schedule's whole job is keeping the PE gap-free from first matmul to the
end. Measured fixed costs that shape the schedule: ~6us engine preamble,
~2.6us DMA ring-fetch latency from issue to data, ~0.7us semaphore lag,
~8.7us end-of-context closure.
  - Blocks: seven full 512-token blocks then two 256-token blocks, so
    the final flush/copy/store drain is short.
  - DMA issue order == compute-need order, two rings with disjoint jobs:
    gpsimd ring carries all w12 gate/up chunks in ascending-hh order and
    then the bulk token blocks 2-7 (in-ring order keeps the weight
    stream ahead of the bulk HBM traffic); sync carries x block 0/1, w3,
    and the in-loop stores.
  - The last block's stores go as two do-pair DMAs on sync + scalar (NOT
    gpsimd: the SWDGE ring takes a ~4us software drain after its last
    DMA, which would land on the end-of-kernel closure path).
"""

import os

import numpy as np
import ml_dtypes

import concourse.bass as bass
import concourse.mybir as mybir
import concourse.tile as tile
from concourse import bacc
from concourse.bass_utils import run_bass_kernel_spmd

N_CORES = 8
D = 512  # d_model
H = 1408  # hidden
TWOH = 2 * H
TPE = 4096  # tokens per expert
NT = 512  # full token block (one PSUM bank in f32)
KD = D // 128  # 4 contraction tiles over d
KH = H // 128  # 11 contraction tiles over h

# (start, size) token blocks: full middles, small tail (see docstring)
BLOCKS = [(512 * b, 512) for b in range(7)] + [(3584, 256), (3840, 256)]

F16 = mybir.dt.float16
F32 = mybir.dt.float32
NP_F16 = np.dtype(np.float16)

# Results of a traced run (test harness reads these).
last_exec_time_ns = None
last_trace_path = None


def _build():
    # Bacc (not plain Bass): its compile() pass pipeline legalizes sync
    # waits (>=2 waits per instruction are split into event-sem chains),
    # which this image's walrus requires.
    nc = bacc.Bacc("TRN2", target_bir_lowering=False, debug=False, num_devices=N_CORES)
    xt = nc.dram_tensor("xt", [D, TPE], F16, kind="ExternalInput")
    w12t = nc.dram_tensor("w12t", [D, TWOH], F16, kind="ExternalInput")
    w3t = nc.dram_tensor("w3t", [H, D], F16, kind="ExternalInput")
    outT = nc.dram_tensor("outT", [D, TPE], F32, kind="ExternalOutput")

    # GEMM2 is software-pipelined into the GEMM1/SwiGLU loop with this lag:
    # in iteration hh we issue the GEMM2 matmuls consuming ht[hh - LAG], so
    # the PE never waits on the ACT+DVE SwiGLU chain.
    LAG = 2

    with tile.TileContext(nc) as tc:
        with (
            tc.tile_pool(name="weights", bufs=1) as wpool,
            tc.tile_pool(name="xin", bufs=1) as xpool,
            tc.tile_pool(name="ht", bufs=2) as hpool,
            tc.tile_pool(name="swi", bufs=4) as spool,
            tc.tile_pool(name="ot", bufs=4) as opool,
            tc.tile_pool(name="pg", bufs=2, space=bass.MemorySpace.PSUM) as pgate,
            tc.tile_pool(name="pu", bufs=2, space=bass.MemorySpace.PSUM) as pup,
            tc.tile_pool(name="po", bufs=1, space=bass.MemorySpace.PSUM) as pacc,
        ):
            w12s = wpool.tile([128, KD, TWOH], F16)
            w3s = wpool.tile([128, KH, D], F16)
            xs = xpool.tile([128, KD, TPE], F16)

            xt_r = xt[:, :].rearrange("(kd p) t -> p kd t", p=128)
            w12_r = w12t[:, :].rearrange("(kd p) h -> p kd h", p=128)
            w3_r = w3t[:, :].rearrange("(kh p) d -> p kh d", p=128)

            def dma_w12(c0, c1, eng=None):
                (eng or nc.gpsimd).dma_start(
                    out=w12s[:, :, c0:c1], in_=w12_r[:, :, c0:c1]
                )

            def dma_x(t0, t1, eng):
                eng.dma_start(out=xs[:, :, t0:t1], in_=xt_r[:, :, t0:t1])

            # DMA issue order == compute-need order (see module docstring):
            # all w12 chunks in ascending-hh order on the gpsimd ring with
            # the bulk token loads trailing them (in-ring order keeps the
            # weight stream ahead of bulk HBM traffic); x block 0/1 and w3
            # on the sync ring.
            dma_x(0, 512, nc.sync)
            dma_w12(0, 128)  # gate hh=0: first-matmul critical
            dma_w12(H, H + 128)
            nc.sync.dma_start(out=w3s[:, 0:1, :], in_=w3_r[:, 0:1, :])
            for c0, c1 in [(128, 256), (256, 512), (512, 768), (768, 1088), (1088, 1408)]:
                dma_w12(c0, c1)
                dma_w12(H + c0, H + c1)
            dma_x(512, 1024, nc.sync)
            nc.sync.dma_start(out=w3s[:, 1:6, :], in_=w3_r[:, 1:6, :])
            nc.sync.dma_start(out=w3s[:, 6:KH, :], in_=w3_r[:, 6:KH, :])
            # Bulk token blocks trail the weight stream on the gpsimd ring.
            dma_x(1024, 2560, nc.gpsimd)
            dma_x(2560, 4096, nc.gpsimd)

            outT_r = outT[:, :].rearrange("(do p) t -> p do t", p=128)

            for tb, (ts0, bs) in enumerate(BLOCKS):
                tsl = slice(ts0, ts0 + bs)
                ht = hpool.tile([128, KH, NT], F16)
                acc = [
                    pacc.tile([128, NT], F32, name=f"acc{do}", tag=f"acc{do}")
                    for do in range(KD)
                ]

                def gemm2_step(kh):
                    for do in range(KD):
                        nc.tensor.matmul(
                            acc[do][:, :bs],
                            w3s[:, kh, do * 128 : (do + 1) * 128],
                            ht[:, kh, :bs],
                            start=(kh == 0),
                            stop=(kh == KH - 1),
                        )

                for hh in range(KH):
                    ps_g = pgate.tile([128, NT], F32)
                    ps_u = pup.tile([128, NT], F32)
                    for ps, coff in ((ps_g, 0), (ps_u, H)):
                        for kd in range(KD):
                            nc.tensor.matmul(
                                ps[:, :bs],
                                w12s[:, kd, coff + hh * 128 : coff + (hh + 1) * 128],
                                xs[:, kd, tsl],
                                start=(kd == 0),
                                stop=(kd == KD - 1),
                            )
                    sil = spool.tile([128, NT], F32)
                    nc.scalar.activation(
                        sil[:, :bs], ps_g[:, :bs], mybir.ActivationFunctionType.Silu
                    )
                    nc.vector.tensor_mul(ht[:, hh, :bs], sil[:, :bs], ps_u[:, :bs])
                    if hh >= LAG:
                        gemm2_step(hh - LAG)
                for kh in range(KH - LAG, KH):
                    gemm2_step(kh)

                # PSUM->SBUF copies split across ACT and DVE; one coalesced
                # output DMA per block, except the last block where two
                # do-pair stores issue on sync + scalar in parallel.
                ot = opool.tile([128, KD, NT], F32)
                for do in range(KD):
                    if do % 2 == 0:
                        nc.scalar.copy(ot[:, do, :bs], acc[do][:, :bs])
                    else:
                        nc.vector.tensor_copy(ot[:, do, :bs], acc[do][:, :bs])
                if tb < len(BLOCKS) - 1:
                    nc.sync.dma_start(out=outT_r[:, :, tsl], in_=ot[:, :, :bs])
                else:
                    nc.sync.dma_start(out=outT_r[:, 0:2, tsl], in_=ot[:, 0:2, :bs])
                    nc.scalar.dma_start(
                        out=outT_r[:, 2:4, tsl], in_=ot[:, 2:4, :bs]
                    )
    nc.compile()
    return nc


_nc_cache = None


def _get_nc():
    global _nc_cache
    if _nc_cache is None:
        _nc_cache = _build()
    return _nc_cache


def kernel(sorted_x, w12, w3, expert_starts, expert_ends):
    global last_exec_time_ns, last_trace_path
    sorted_x = np.asarray(sorted_x)
    w12 = np.asarray(w12)
    w3 = np.asarray(w3)
    starts = np.asarray(expert_starts).astype(np.int64)
    T = sorted_x.shape[0]

    in_maps = []
    for e in range(N_CORES):
        # jax.lax.dynamic_slice clamps the start index the same way
        s = int(min(max(starts[e], 0), T - TPE))
        xe = sorted_x[s : s + TPE]  # (TPE, D) f32
        in_maps.append(
            {
                "xt": np.ascontiguousarray(xe.T).astype(NP_F16),
                "w12t": np.ascontiguousarray(w12[e].T).astype(NP_F16),
                "w3t": np.ascontiguousarray(w3[e].T).astype(NP_F16),
            }
        )

    trace = bool(os.environ.get("BASS_MOE_TRACE"))
    res = run_bass_kernel_spmd(
        _get_nc(), in_maps, core_ids=list(range(N_CORES)), trace=trace
    )
    if trace:
        last_exec_time_ns = res.exec_time_ns
        iat = res.instructions_and_trace
        last_trace_path = iat[1] if iat else None

    out = np.empty((N_CORES * TPE, D), dtype=np.float32)
    for e in range(N_CORES):
        out[e * TPE : (e + 1) * TPE] = res.results[e]["outT"].T
    return out
